# revision 63
# baseline (speedup 1.0000x reference)
"""GroupedQueryAttention Trainium2 kernel (8 NeuronCores), v3.

Sharding: core i handles (batch b = i//4, KV group g = i%4): its 4 query
heads + 1 KV group, full sequence. Each core computes a partial output
(attn_heads @ Wo rows for its heads) in bf16; host sums the 4 partials
per batch in fp32.

v3 strategy (per core), all PE inputs bf16 (PSUM accum fp32):
  - warmup: ~64 junk matmuls release the HAM clock gate (2.4GHz) while
    the DMA subsystem boots (~8us before first delivery).
  - projections: W-stationary matmuls over 16 e-chunks, moving x bf16;
    x streamed on the gpsimd SWDGE queue, wk/wv/consts on sync, wq on
    scalar, ordered by first use.
  - RoPE: host permutes W rows to half-split layout; half-swap via
    SBUF->SBUF DMA on the sync queue for tb0-2 (off critical path) and
    via PE permutation matmul for tb3; cos/sin multiply-add on DVE.
  - attention per (bi, h) unit: j-pairs share a [128,1024] psum tile
    (two 512-wide score matmuls), batched exp on ACT; causal masks on
    DVE/gpsimd (split per bi by engine budget); softmax denominators:
    diagonal pairs via narrow ones-matmuls on PE, full-width pairs
    accumulated at producer time into two sbuf accumulators (DVE chain +
    gpsimd chain, independent), folded by ones-matmuls; unit
    finalization (fold + reciprocal + normalize) deferred one unit so
    PE never waits on the accumulation chains; the block's last unit
    normalizes in 128-col chunks so the out-projection starts early.
  - out projection: per 128-row tq tile, 4x 512-col accumulation chains
    into two [128,1024] psum tiles, drained by ACT+DVE to bf16, output
    DMA halves round-robined over sync/scalar/gpsimd queues (gpsimd
    excluded for the last block so its SWDGE queue drains early; final
    tile split into quarter copies/DMAs for a short tail).
PSUM: exactly 2 pools x [128,1024] f32 x 2 bufs = 8 banks.

v3.1: tb0 projects all six chains in one e-pass (3 open psum tiles) so the
DMA ramp is demand-paced, not burst-starved; tb3 ropes k/q0 between the
pq23 chains and defers its vsd drain (needed only by bi3); the final
output tile copies+DMAs each eo quarter as soon as its chain closes.
Measured: ~255us median (baseline 271.7us), rel err 6.8e-3.
"""

import numpy as np
import ml_dtypes
from contextlib import ExitStack

import concourse.bass as bass
import concourse.bacc as bacc
import concourse.tile as tile
import concourse.mybir as mybir
from concourse.bass_utils import run_bass_kernel_spmd

# problem shape (hardcoded per contract)
B, T, E = 2, 2048, 2048
NH, NG, HD = 16, 4, 128
HPG = NH // NG          # 4 heads per group = per core
NE = E // 128           # 16 contraction chunks
TB = 512                # tq / t block
NTB = T // TB           # 4
F32 = mybir.dt.float32
BF16 = mybir.dt.bfloat16
EXP = mybir.ActivationFunctionType.Exp
NPBF16 = ml_dtypes.bfloat16

N_CORES = 8


def build_body(tc, out_ap, ins):
    """ins: dict name -> dram AP. out_ap: [T, E] dram AP (bf16)."""
    nc = tc.nc
    ctx = ExitStack()
    with ctx:
        ctx.enter_context(nc.allow_low_precision(
            reason="bf16 matmul inputs / outputs are intended"))

        # ---- constant / persistent SBUF ----
        const = ctx.enter_context(tc.tile_pool(name="const", bufs=1))
        cs2 = const.tile([128, T], BF16, tag="cs2", name="cs2")
        snpm = const.tile([128, T], BF16, tag="snpm", name="snpm")
        tri = const.tile([128, 128], BF16, tag="tri", name="tri")
        swp = const.tile([128, 128], BF16, tag="swp", name="swp")
        iden = const.tile([128, 128], F32, tag="iden", name="iden")
        ones = const.tile([128, 128], BF16, tag="ones", name="ones")

        persist = ctx.enter_context(tc.tile_pool(name="persist", bufs=1))
        # packed x: [128, tb*8192 + e*512 + c]
        xb = persist.tile([128, NTB * NE * TB], BF16, tag="xb", name="xb")
        big = persist.tile([128, 6 * T], BF16, tag="big", name="big")
        qrot = [big[:, h * T:(h + 1) * T] for h in range(HPG)]
        krot = big[:, 4 * T:5 * T]
        vsd = big[:, 5 * T:6 * T]
        aout = qrot  # attn output overwrites qrot block-by-block

        # ---- weights (packed by host into sbuf layout) ----
        wpool = ctx.enter_context(tc.tile_pool(name="weights", bufs=1))
        wq_t = wpool.tile([128, NE * 512], BF16, tag="wq", name="wq")
        wk_t = wpool.tile([128, NE * 128], BF16, tag="wk", name="wk")
        wv_t = wpool.tile([128, NE * 128], BF16, tag="wv", name="wv")
        wo_t = wpool.tile([128, NE * 512], BF16, tag="wo", name="wo")

        # ---- psum pools: 2 pools x [128,1024] x 2 bufs = 8 banks ----
        PB = ctx.enter_context(tc.tile_pool(name="pb", bufs=2, space="PSUM"))
        PD = ctx.enter_context(tc.tile_pool(name="pd", bufs=2, space="PSUM"))

        # ---- sbuf working pools ----
        rawp = ctx.enter_context(tc.tile_pool(name="rawp", bufs=3))
        ptp = ctx.enter_context(tc.tile_pool(name="ptp", bufs=4))
        rdp = ctx.enter_context(tc.tile_pool(name="rdp", bufs=2))
        osp = ctx.enter_context(tc.tile_pool(name="osp", bufs=2))
        accp = ctx.enter_context(tc.tile_pool(name="accp", bufs=6))

        # ---------------- DMA preload ----------------
        # Each issuing engine owns a ~140GB/s DMA queue; split the load three
        # ways and order by first use so the e-interleaved projection streams
        # against DMA arrival.  (The DMA subsystem only starts delivering at
        # ~8us; fine-grained chunks only add per-descriptor overhead.)
        # warmup source tile: memset first on gpsimd, ahead of its DMA work
        junk = const.tile([128, 64], BF16, tag="junk", name="junk")
        nc.gpsimd.memset(junk[:], 0.0)
        # gpsimd queue: x, progressive chunks per tb.  tb0: fine 2e chunks up
        # front for the ramp, coarser tail so tb1's first chunk isn't delayed
        # by SWDGE per-descriptor overhead.
        for e0, e1 in ((0, 2), (2, 4), (4, 6), (6, 8), (8, 12), (12, 16)):
            nc.gpsimd.dma_start(xb[:, e0 * TB:e1 * TB],
                                ins["xb"][:, e0 * TB:e1 * TB])
        for tb in range(1, NTB):
            for c in range(4):
                w = NE * TB // 4
                base = tb * NE * TB + c * w
                nc.gpsimd.dma_start(xb[:, base:base + w],
                                    ins["xb"][:, base:base + w])
        # sync queue: iden (warmup), wk/wv interleaved quarters, consts, wo;
        # later the rope-swap DMAs and output tiles.
        nc.sync.dma_start(iden[:], ins["iden"][:])
        for a, b in ((0, 4), (4, 8), (8, 16)):
            nc.sync.dma_start(wk_t[:, a * 128:b * 128],
                              ins["wk"][:, a * 128:b * 128])
            nc.sync.dma_start(wv_t[:, a * 128:b * 128],
                              ins["wv"][:, a * 128:b * 128])
        nc.sync.dma_start(tri[:], ins["tri"][:])
        nc.sync.dma_start(ones[:], ins["ones"][:])
        # wo on gpsimd after x: sync must stay free for tb0/1 rope-swap DMAs
        for c in range(4):
            nc.gpsimd.dma_start(wo_t[:, c * 4 * 512:(c + 1) * 4 * 512],
                                ins["wo"][:, c * 4 * 512:(c + 1) * 4 * 512])
        # scalar queue: swp, wq progressive by 2e, rope tables.
        nc.scalar.dma_start(swp[:], ins["swp"][:])
        for c in range(8):
            nc.scalar.dma_start(wq_t[:, c * 2 * 512:(c + 1) * 2 * 512],
                                ins["wq"][:, c * 2 * 512:(c + 1) * 2 * 512])
        nc.scalar.dma_start(cs2[:], ins["cs2"][:])
        nc.scalar.dma_start(snpm[:], ins["snpm"][:])

        # PE warmup: the DMA subsystem only delivers from ~8us, so matmul a
        # locally memset tile back-to-back until then. Releases the HAM
        # clock-gate so the first real matmuls run at 2.4GHz.
        warm = PB.tile([128, 1024], F32, tag="b", name="warm")
        for _ in range(96):
            nc.tensor.matmul(warm[0:64, 0:64], junk[:, 0:64], junk[:, 0:64],
                             start=True, stop=True)

        def xc(tb, e):
            base = tb * NE * TB + e * TB
            return xb[:, base:base + TB]

        def rope_copy(ps, tag):
            """Drain psum projection [128, TB] to a bf16 sbuf tile on ACT."""
            raw = rawp.tile([128, TB], BF16, tag="raw", bufs=6, name=f"raw{tag}")
            nc.scalar.copy(raw[:], ps)
            return raw

        def rope_mul(dst_ap, raw, sw, cols, tag):
            tmp1 = rawp.tile([128, TB], BF16, tag="tmp", bufs=2, name=f"t1{tag}")
            tmp2 = rawp.tile([128, TB], BF16, tag="tmp", bufs=2, name=f"t2{tag}")
            nc.vector.tensor_mul(tmp1[:], raw[:], cs2[:, cols])
            nc.vector.tensor_mul(tmp2[:], sw[:], snpm[:, cols])
            nc.vector.tensor_add(dst_ap, tmp1[:], tmp2[:])

        def rope_finish(dst_ap, raw, psw_half, cols, tag):
            """dst = raw*cos + swap(raw)*sgn_sin; swap via PE perm matmul.
            (Only used for tb3; DVE drains psum there since ACT is the
            bottleneck at the projection->attention transition.)"""
            nc.tensor.matmul(psw_half, swp[:], raw[:], start=True, stop=True)
            sw = rawp.tile([128, TB], BF16, tag="sw", name=f"sw{tag}")
            nc.vector.tensor_copy(sw[:], psw_half)
            rope_mul(dst_ap, raw, sw, cols, tag)

        def rope_finish_dma(dst_ap, raw, cols, tag):
            """Half-swap via SBUF->SBUF DMA on the (idle) sync queue instead
            of a PE matmul; fine off the critical path (tb0-2)."""
            sw = rawp.tile([128, TB], BF16, tag="sw", name=f"sw{tag}")
            nc.sync.dma_start(sw[0:64, :], raw[64:128, :])
            nc.sync.dma_start(sw[64:128, :], raw[0:64, :])
            rope_mul(dst_ap, raw, sw, cols, tag)

        def jmeta(bi):
            jorder = list(range(4 * bi, 4 * bi + 4)) + list(range(4 * bi))
            return [(jorder[2 * p], jorder[2 * p + 1])
                    for p in range(len(jorder) // 2)]

        def off(bi, j):
            return 128 * (j - 4 * bi) if j >= 4 * bi else 0

        acc_of = {}

        def scores_exp(bi, h, jp):
            j0, j1 = jmeta(bi)[jp]
            o0, o1 = off(bi, j0), off(bi, j1)
            pb = PB.tile([128, 1024], F32, tag="b", name="pb")
            nc.tensor.matmul(
                pb[:, 0:TB - o0],
                krot[:, j0 * 128:(j0 + 1) * 128],
                qrot[h][:, bi * TB + o0:(bi + 1) * TB],
                start=True, stop=True)
            nc.tensor.matmul(
                pb[:, TB:2 * TB - o1],
                krot[:, j1 * 128:(j1 + 1) * 128],
                qrot[h][:, bi * TB + o1:(bi + 1) * TB],
                start=True, stop=True)
            pt = ptp.tile([128, 1024], BF16, tag="pt", name="pt")
            if o0 == 0 and o1 == 0:
                nc.scalar.activation(pt[:], pb[:], EXP)
            else:
                nc.scalar.activation(pt[:, o0:TB], pb[:, 0:TB - o0], EXP)
                nc.scalar.activation(pt[:, TB + o1:2 * TB],
                                     pb[:, TB:2 * TB - o1], EXP)
            for half, j, o in ((0, j0, o0), (1, j1, o1)):
                if j >= 4 * bi:  # diagonal tile: causal mask.  DVE where its
                    # budget allows (low latency); gp at bi0 (half) / bi3.
                    gp_mask = (bi == 0 and half == 1) or bi == 3
                    meng = nc.gpsimd if gp_mask else nc.vector
                    c0 = half * TB
                    meng.tensor_mul(pt[:, c0 + o:c0 + o + 128],
                                    pt[:, c0 + o:c0 + o + 128], tri[:])
            # Denominator accumulation for full-width pairs, at producer time
            # (AHEAD slots early).  Two independent accumulators: gp chain
            # (early pairs) + DVE chain (late pairs) -> no cross-engine
            # serialization; both folded by ones-matmuls in finish_unit.
            if 2 <= jp:
                gp_hi = {1: 1, 2: 3, 3: 3}[bi]  # gp takes jp in [2, gp_hi]
                eng, key = (nc.gpsimd, (bi, h, 'g')) if jp <= gp_hi else \
                           (nc.vector, (bi, h, 'v'))
                if key not in acc_of:
                    acc = accp.tile([128, TB], BF16, tag="acc", name="acc")
                    acc_of[key] = acc
                    eng.tensor_add(acc[:], pt[:, 0:TB], pt[:, TB:2 * TB])
                else:
                    acc = acc_of[key]
                    eng.tensor_add(acc[:], acc[:], pt[:, 0:TB])
                    eng.tensor_add(acc[:], acc[:], pt[:, TB:2 * TB])
            return pt

        def denom_pv(bi, h, jp, pa, pt):
            """Denominator: diagonal pairs (jp<2, narrow) via ones-matmul on
            PE; full-width pairs were accumulated on DVE at producer time,
            folded by a single ones-matmul in finish_unit. PV unchanged."""
            j0, j1 = jmeta(bi)[jp]
            o0, o1 = off(bi, j0), off(bi, j1)
            last = (jp == 2 * bi + 1)
            if jp < 2:
                nc.tensor.matmul(pa[:, TB + o0:2 * TB], ones[:],
                                 pt[:, o0:TB], start=(jp == 0), stop=False)
                nc.tensor.matmul(pa[:, TB + o1:2 * TB], ones[:],
                                 pt[:, TB + o1:2 * TB], start=False,
                                 stop=(last and bi == 0))
            nc.tensor.matmul(pa[:, o0:TB], vsd[:, j0 * 128:(j0 + 1) * 128],
                             pt[:, o0:TB], start=(jp == 0), stop=False)
            nc.tensor.matmul(pa[:, o1:TB], vsd[:, j1 * 128:(j1 + 1) * 128],
                             pt[:, TB + o1:2 * TB], start=False, stop=last)

        from collections import deque
        ptq = deque()
        AHEAD = 2
        all_units = {bi: [(h, jp) for h in range(HPG) for jp in range(2 * bi + 2)]
                     for bi in range(NTB)}

        def prologue():
            for k in range(AHEAD):
                h, jp = all_units[0][k]
                ptq.append(scores_exp(0, h, jp))

        # ================= projection phase =================
        # First pass interleaves chains k, v, q0, q1 by e-chunk so the PE
        # consumes x/wq in DMA arrival order (no big re-scan stalls on tb0);
        # q2/q3 run as a second pass (into a PB tile) over the by-then
        # resident x block while ACT drains the first-pass psums. RoPE is
        # split into ACT copy-out (right after each chain) and the
        # swap-matmul/DVE math (emitted after the q2/q3 chains) so the PE
        # never waits on ACT.
        for tb in range(NTB):
            cols = slice(tb * TB, (tb + 1) * TB)
            tb_last = (tb == NTB - 1)
            pkv = PD.tile([128, 1024], F32, tag="d", name="pkv")
            pq01 = PD.tile([128, 1024], F32, tag="d", name="pq01")
            if tb == 0:
                # tb0: all six chains in ONE e-pass (3 open psum tiles).
                # Halves the per-e x/weight DMA demand during the ramp,
                # where all queues stream simultaneously.
                pq23 = PB.tile([128, 1024], F32, tag="b", name="pq23")
            for e in range(NE):
                st, sp = (e == 0), (e == NE - 1)
                nc.tensor.matmul(pkv[:, 0:TB], wk_t[:, e * 128:(e + 1) * 128],
                                 xc(tb, e), start=st, stop=sp)
                nc.tensor.matmul(pkv[:, TB:2 * TB], wv_t[:, e * 128:(e + 1) * 128],
                                 xc(tb, e), start=st, stop=sp)
                for k in range(2):
                    nc.tensor.matmul(
                        pq01[:, k * TB:(k + 1) * TB],
                        wq_t[:, e * 512 + k * 128: e * 512 + (k + 1) * 128],
                        xc(tb, e), start=st, stop=sp)
                if tb == 0:
                    for k in range(2):
                        nc.tensor.matmul(
                            pq23[:, k * TB:(k + 1) * TB],
                            wq_t[:, e * 512 + (2 + k) * 128:
                                 e * 512 + (3 + k) * 128],
                            xc(tb, e), start=st, stop=sp)
            raw_k = rope_copy(pkv[:, 0:TB], "k")
            vtmp = rawp.tile([128, TB], F32, tag="vtmp", bufs=2, name="vtmp")
            nc.scalar.copy(vtmp[:], pkv[:, TB:2 * TB])
            raw_q0 = rope_copy(pq01[:, 0:TB], "q0")
            raw_q1 = rope_copy(pq01[:, TB:2 * TB], "q1")

            if tb > 0:
                pq23 = PB.tile([128, 1024], F32, tag="b", name="pq23")
            for k in (() if tb == 0 else range(2)):
                h = 2 + k
                for e in range(NE):
                    nc.tensor.matmul(
                        pq23[:, k * TB:(k + 1) * TB],
                        wq_t[:, e * 512 + h * 128: e * 512 + (h + 1) * 128],
                        xc(tb, e), start=(e == 0), stop=(e == NE - 1))
                if k == 0 and tb_last:
                    # rope k/q0 between the pq23 chains: their raws landed
                    # while pq01/pkv drained, so the swap matmuls run
                    # immediately and DVE ropes them under the h3 chain.
                    raw_q2 = rope_copy(pq23[:, 0:TB], "q2")
                    psw1 = PB.tile([128, 1024], F32, tag="b", name="psw1")
                    rope_finish(krot[:, cols], raw_k, psw1[:, 0:TB], cols, "k")
                    rope_finish(qrot[0][:, cols], raw_q0,
                                psw1[:, TB:2 * TB], cols, "q0")
            if tb_last:
                raw_q3 = rope_copy(pq23[:, TB:2 * TB], "q3")
                psw3 = PD.tile([128, 1024], F32, tag="d", name="psw3")
                for jj in range(4):
                    nc.tensor.transpose(
                        psw3[:, TB + jj * 128:TB + (jj + 1) * 128],
                        vtmp[:, jj * 128:(jj + 1) * 128], iden[:])
                prologue()
                psw2 = PD.tile([128, 1024], F32, tag="d", name="psw2")
                rope_finish(qrot[1][:, cols], raw_q1, psw2[:, 0:TB], cols, "q1")
                rope_finish(qrot[2][:, cols], raw_q2, psw2[:, TB:2 * TB], cols, "q2")
                rope_finish(qrot[3][:, cols], raw_q3, psw3[:, 0:TB], cols, "q3")
                # tb3's vsd feeds only bi3, ~100us away: drain late on ACT
                nc.scalar.copy(vsd[:, cols], psw3[:, TB:2 * TB])
            else:
                raw_q2 = rope_copy(pq23[:, 0:TB], "q2")
                raw_q3 = rope_copy(pq23[:, TB:2 * TB], "q3")
                psw3 = PD.tile([128, 1024], F32, tag="d", name="psw3")
                for jj in range(4):
                    nc.tensor.transpose(
                        psw3[:, TB + jj * 128:TB + (jj + 1) * 128],
                        vtmp[:, jj * 128:(jj + 1) * 128], iden[:])
                nc.vector.tensor_copy(vsd[:, cols], psw3[:, TB:2 * TB])
                # off the critical path: swap via DMA, no PE/PSUM involved
                rope_finish_dma(qrot[3][:, cols], raw_q3, cols, "q3")
                rope_finish_dma(qrot[1][:, cols], raw_q1, cols, "q1")
                rope_finish_dma(qrot[2][:, cols], raw_q2, cols, "q2")
                rope_finish_dma(krot[:, cols], raw_k, cols, "k")
                rope_finish_dma(qrot[0][:, cols], raw_q0, cols, "q0")


        # ================= attention + out-projection =================
        # Flat software pipeline: the scores+exp producer runs AHEAD units
        # in front of the denominator/PV consumer, across head and bi-block
        # boundaries (the next block's first two units are emitted before
        # this block's out-projection so ACT exps while PE projects).
        oq = [nc.sync, nc.scalar, nc.gpsimd]
        oqi = [0]

        def out_dma(dst, src, ngq=3):
            # ngq=2 keeps gpsimd out so its SWDGE queue drains before the end
            oq[oqi[0] % ngq].dma_start(dst, src)
            oqi[0] += 1
        # Unit finalization (denominator fold + normalize) is deferred by one
        # unit so the DVE/gpsimd accumulation chain never stalls the PE.
        pend = [None]

        def finish_unit():
            if pend[0] is None:
                return
            fbi, fh, fpa = pend[0]
            pend[0] = None
            if fbi > 0:
                accs = [a for a in (acc_of.pop((fbi, fh, 'g'), None),
                                    acc_of.pop((fbi, fh, 'v'), None))
                        if a is not None]
                for n, facc in enumerate(accs):
                    nc.tensor.matmul(fpa[:, TB:2 * TB], ones[:], facc[:],
                                     start=False, stop=(n == len(accs) - 1))
            rden = rdp.tile([128, TB], F32, tag="rden", name="rden")
            if fh == HPG - 1:
                # last unit of the block gates the out-projection: normalize
                # in 128-col chunks so out-proj tq0 starts after chunk 0.
                for cq in range(4):
                    c = slice(cq * 128, (cq + 1) * 128)
                    fc = slice(fbi * TB + cq * 128, fbi * TB + (cq + 1) * 128)
                    nc.vector.reciprocal_approx_fast(rden[:, c],
                                                     fpa[:, TB + cq * 128:TB + (cq + 1) * 128])
                    nc.vector.tensor_mul(aout[fh][:, fc], fpa[:, c], rden[:, c])
            else:
                fq = slice(fbi * TB, (fbi + 1) * TB)
                nc.vector.reciprocal_approx_fast(rden[:], fpa[:, TB:2 * TB])
                nc.vector.tensor_mul(aout[fh][:, fq], fpa[:, 0:TB], rden[:])

        for bi in range(NTB):
            units = all_units[bi]
            pa = None
            for i, (h, jp) in enumerate(units):
                if i + AHEAD < len(units):
                    h2, jp2 = units[i + AHEAD]
                    ptq.append(scores_exp(bi, h2, jp2))
                elif bi + 1 < NTB:
                    h2, jp2 = all_units[bi + 1][i + AHEAD - len(units)]
                    ptq.append(scores_exp(bi + 1, h2, jp2))
                if jp == 0:
                    finish_unit()
                    pa = PD.tile([128, 1024], F32, tag="d", name="pa")
                denom_pv(bi, h, jp, pa, ptq.popleft())
                if jp == 2 * bi + 1:
                    pend[0] = (bi, h, pa)
            finish_unit()

            # out-projection for this bi block; output DMA halves round-robin
            # across all four queues so the final block drains in parallel.
            for tq in range(4):
                trows = slice(bi * TB + tq * 128, bi * TB + (tq + 1) * 128)
                po = [PD.tile([128, 1024], F32, tag="d", name="po") for _ in range(2)]
                final = (bi == NTB - 1 and tq == 3)
                osb = osp.tile([128, 2048], BF16, tag="osb", name="osb")
                ngq = 2 if bi == NTB - 1 else 3
                if tq == 0:
                    # first row-tile is gated by the last unit's normalize on
                    # DVE: run the 12 finish-independent hh=0-2 matmuls first,
                    # deferring the four hh=3 matmuls (accumulation into a
                    # psum region is order-independent).
                    for eo in range(4):
                        tgt = po[eo // 2][:, (eo % 2) * TB:(eo % 2 + 1) * TB]
                        for hh in range(HPG - 1):
                            nc.tensor.matmul(
                                tgt, aout[hh][:, trows],
                                wo_t[:, (hh * 4 + eo) * 512:
                                     (hh * 4 + eo + 1) * 512],
                                start=(hh == 0), stop=False)
                    for eo in range(4):
                        tgt = po[eo // 2][:, (eo % 2) * TB:(eo % 2 + 1) * TB]
                        hh = HPG - 1
                        nc.tensor.matmul(
                            tgt, aout[hh][:, trows],
                            wo_t[:, (hh * 4 + eo) * 512:
                                 (hh * 4 + eo + 1) * 512],
                            start=False, stop=True)
                for eo in range(4 if tq > 0 else 0):
                    tgt = po[eo // 2][:, (eo % 2) * TB:(eo % 2 + 1) * TB]
                    for hh in range(HPG):
                        nc.tensor.matmul(
                            tgt, aout[hh][:, trows],
                            wo_t[:, (hh * 4 + eo) * 512:(hh * 4 + eo + 1) * 512],
                            start=(hh == 0), stop=(hh == HPG - 1))
                    if final:
                        # final tile: copy+DMA each eo quarter as soon as its
                        # chain closes, overlapping the remaining chains ->
                        # short tail (copies alternate ACT/DVE; different
                        # psum banks than the chain still accumulating).
                        qc = slice(eo * 512, (eo + 1) * 512)
                        if eo % 2 == 0:
                            nc.scalar.copy(osb[:, qc], tgt)
                        else:
                            nc.vector.tensor_copy(osb[:, qc], tgt)
                        out_dma(out_ap[trows, qc], osb[:, qc], ngq)
                if final:
                    pass
                else:
                    nc.scalar.copy(osb[:, 0:1024], po[0][:])
                    nc.vector.tensor_copy(osb[:, 1024:2048], po[1][:])
                    out_dma(out_ap[trows, 0:1024], osb[:, 0:1024], ngq)
                    out_dma(out_ap[trows, 1024:2048], osb[:, 1024:2048], ngq)


# ---------------- host side ----------------

_PERM = np.concatenate([np.arange(0, HD, 2), np.arange(1, HD, 2)])  # half-split


def _pack_w(w):
    """[E, C] -> [128, NE*C] sbuf layout (col block = e-chunk)."""
    c = w.shape[1]
    return np.ascontiguousarray(
        w.reshape(NE, 128, c).transpose(1, 0, 2).reshape(128, NE * c)
    ).astype(NPBF16)


def host_prep(inputs):
    """Full inputs -> list of 8 per-core input dicts (core i = (b=i//4, g=i%4))."""
    x = np.asarray(inputs["x"], dtype=np.float32)
    Wq = np.asarray(inputs["Wq"], dtype=np.float32)
    Wk = np.asarray(inputs["Wk"], dtype=np.float32)
    Wv = np.asarray(inputs["Wv"], dtype=np.float32)
    Wo = np.asarray(inputs["Wo"], dtype=np.float32)

    inv = (10000.0 ** (-np.arange(0, HD, 2, dtype=np.float32) / HD)).astype(np.float32)
    tpos = np.arange(T, dtype=np.float32)
    fr = np.outer(tpos, inv)                       # [T, 64]
    cosT = np.cos(fr).T.astype(np.float32)         # [64, T]
    sinT = np.sin(fr).T.astype(np.float32)
    cs2 = np.concatenate([cosT, cosT], axis=0).astype(NPBF16)     # [128, T]
    snpm = np.concatenate([-sinT, sinT], axis=0).astype(NPBF16)   # [128, T]

    tri = (np.arange(128)[None, :] >= np.arange(128)[:, None]).astype(NPBF16)
    swp = np.zeros((128, 128), dtype=np.float32)
    swp[(np.arange(128) + 64) % 128, np.arange(128)] = 1.0
    swp = swp.astype(NPBF16)
    iden = np.eye(128, dtype=np.float32)
    ones = np.ones((128, 128), dtype=np.float32).astype(NPBF16)

    scale = np.float32(1.0 / np.sqrt(HD))
    # xb[b]: [128, tb*8192 + e*512 + c] = x[b][tb*512+c, e*128+p]
    xbs = []
    for b in range(B):
        xT = x[b].T                                  # [E, T]
        v = xT.reshape(NE, 128, NTB, TB).transpose(1, 2, 0, 3)
        xbs.append(np.ascontiguousarray(v.reshape(128, NTB * NE * TB)).astype(NPBF16))

    in_maps = []
    for i in range(N_CORES):
        b, g = i // 4, i % 4
        rows = []
        for h in range(HPG):
            base = (g * HPG + h) * HD
            rows.append(Wq[base + _PERM, :])
        wq_c = (np.concatenate(rows, axis=0) * scale).T  # [E, 512]
        wk_c = Wk[g * HD + _PERM, :].T                   # [E, 128]
        wv_c = Wv[g * HD:(g + 1) * HD, :].T              # [E, 128]
        # wo blocks (hh, eo): [128, (hh*4+eo)*512 + c] = WoT[hh*128+p, eo*512+c]
        wo_c = Wo[:, g * 512:(g + 1) * 512].T            # [512, E]
        wo_p = np.ascontiguousarray(
            wo_c.reshape(HPG, 128, 4, 512).transpose(1, 0, 2, 3).reshape(128, NE * 512)
        ).astype(NPBF16)
        in_maps.append({
            "xb": xbs[b],
            "wq": _pack_w(wq_c),
            "wk": _pack_w(wk_c),
            "wv": _pack_w(wv_c),
            "wo": wo_p,
            "cs2": cs2, "snpm": snpm, "tri": tri, "swp": swp, "iden": iden,
            "ones": ones,
        })
    return in_maps


_NC = None


def build_nc():
    global _NC
    if _NC is not None:
        return _NC
    nc = bacc.Bacc("TRN2", target_bir_lowering=False, debug=False,
                   num_devices=N_CORES)
    ins = {
        "xb": nc.dram_tensor("xb", [128, NTB * NE * TB], BF16, kind="ExternalInput").ap(),
        "wq": nc.dram_tensor("wq", [128, NE * 512], BF16, kind="ExternalInput").ap(),
        "wk": nc.dram_tensor("wk", [128, NE * 128], BF16, kind="ExternalInput").ap(),
        "wv": nc.dram_tensor("wv", [128, NE * 128], BF16, kind="ExternalInput").ap(),
        "wo": nc.dram_tensor("wo", [128, NE * 512], BF16, kind="ExternalInput").ap(),
        "cs2": nc.dram_tensor("cs2", [128, T], BF16, kind="ExternalInput").ap(),
        "snpm": nc.dram_tensor("snpm", [128, T], BF16, kind="ExternalInput").ap(),
        "tri": nc.dram_tensor("tri", [128, 128], BF16, kind="ExternalInput").ap(),
        "swp": nc.dram_tensor("swp", [128, 128], BF16, kind="ExternalInput").ap(),
        "iden": nc.dram_tensor("iden", [128, 128], F32, kind="ExternalInput").ap(),
        "ones": nc.dram_tensor("ones", [128, 128], BF16, kind="ExternalInput").ap(),
    }
    out = nc.dram_tensor("out", [T, E], BF16, kind="ExternalOutput").ap()
    with tile.TileContext(nc) as tc:
        build_body(tc, out, ins)
    nc.compile()
    _NC = nc
    return nc


def gather(results):
    """results: list of 8 dicts with 'out' [T, E] bf16 partials -> [B, T, E] f32."""
    out = np.zeros((B, T, E), dtype=np.float32)
    for i in range(N_CORES):
        out[i // 4] += np.asarray(results[i]["out"], dtype=np.float32)
    return out


def kernel(**inputs):
    nc = build_nc()
    in_maps = host_prep(inputs)
    res = run_bass_kernel_spmd(nc, in_maps, core_ids=list(range(N_CORES)))
    return gather(res.results)


if __name__ == "__main__":
    rng = np.random.default_rng(0)
    ins = {
        "x": rng.standard_normal((B, T, E), dtype=np.float32),
        "Wq": rng.standard_normal((E, E), dtype=np.float32) * 0.02,
        "Wk": rng.standard_normal((NG * HD, E), dtype=np.float32) * 0.02,
        "Wv": rng.standard_normal((NG * HD, E), dtype=np.float32) * 0.02,
        "Wo": rng.standard_normal((E, E), dtype=np.float32) * 0.02,
    }
    out = kernel(**ins)
    print(out.shape, out.dtype, np.abs(out).mean())

